# revision 17
# baseline (speedup 1.0000x reference)
"""Trainium2 Bass kernel for PoissonGaussianReadout.

Computation (per reference):
  out[b, n] = elu( sum_c bilinear_sample(x[b, c], mu[n]) * W[n, c] + bias[n] ) + 1

Sharding: data-parallel over batch B=32 across 8 cores (4 images per core).
Every core processes all N=8192 neurons for its 4 images.

Device strategy per core:
  - x is pre-transposed (host) to pixel-major x_t[4096, 4*256] bf16 so that one
    pixel's (b, c) values form a contiguous 2KB row.
  - For each tile of 128 neurons, gather the 4 bilinear corner pixel-rows as
    two overlapping row-pairs (x0, x0+1) at y0 and y1 via dma_gather
    (int16 indices, elem_size=2048, elem_step=1024) -> G[128, y2, x2, b, c].
  - V[n, k, c] = w_k[n] * W[n, c] is host-precomputed (bf16, resident 16MB).
  - Per b: one fused tensor_tensor_reduce: accum z[n, b] = bias[n] +
    sum_{k,c} G * V.
  - Epilogue: out = exp(min(z,0)) + max(z,0)  (== elu(z) + 1).
"""

import dataclasses

import numpy as np
import ml_dtypes

B, C, H, Wd, N = 32, 256, 64, 64, 8192
NCORES = 8
BL = B // NCORES          # 4 images per core
P = 128                   # partitions / neurons per tile
NT = N // P               # 64 neuron tiles
ROW = BL * C              # 1024 elements per pixel row
NPIX = H * Wd             # 4096

GATHER = "ant"            # "ant" (dma_gather) or "indirect"
COMPUTE = "stt"           # "stt" (scalar_tensor_tensor; "ttr" crashes HW)
TG = 2                    # tiles per dma_gather instruction
GBUFS = 2                 # gather pool buffers
NQUEUES = 1               # SWDGE queues

_PROGRAM = None


def _build_program(nt=NT, gather=None, compute=None, tg=None, gbufs=None,
                   nqueues=None, split_waits=True):
    import concourse.bass as bass
    import concourse.mybir as mybir
    import concourse.tile as tile

    gather = gather or GATHER
    compute = compute or COMPUTE
    tg = tg or TG
    gbufs = gbufs or GBUFS
    nqueues = nqueues or NQUEUES

    bf16 = mybir.dt.bfloat16
    f32 = mybir.dt.float32
    i32 = mybir.dt.int32
    i16 = mybir.dt.int16

    nc = bass.Bass("TRN2", num_swdge_queues=nqueues)

    xt = nc.dram_tensor("xt", [NPIX, ROW], bf16, kind="ExternalInput")
    # V[n, k, c] = corner_weight_k[n] * W[n, c], host-precomputed, bf16
    wv = nc.dram_tensor("wv", [P, nt * 4 * C], bf16, kind="ExternalInput")
    biasr = nc.dram_tensor("biasr", [P, nt], f32, kind="ExternalInput")
    out = nc.dram_tensor("out", [P, nt * BL], f32, kind="ExternalOutput")
    if gather == "ant":
        idx16 = nc.dram_tensor("idx16", [P, nt * 16], i16, kind="ExternalInput")
    else:
        idx = nc.dram_tensor("idx", [P, nt * 2], i32, kind="ExternalInput")

    assert nt % tg == 0

    with tile.TileContext(nc) as tc:
        with (
            tc.tile_pool(name="const", bufs=1) as cpool,
            tc.tile_pool(name="gpool", bufs=gbufs) as gpool,
            tc.tile_pool(name="work", bufs=3) as wpool,
        ):
            if gather == "ant":
                from concourse import library_config
                nc.gpsimd.load_library(library_config.mlp)
            v_sb = cpool.tile([P, nt * 4 * C], bf16)
            nc.sync.dma_start(v_sb[:], wv[:])
            bias_sb = cpool.tile([P, nt], f32)
            nc.sync.dma_start(bias_sb[:], biasr[:])
            if gather == "ant":
                idx_sb = cpool.tile([P, nt * 16], i16)
                nc.sync.dma_start(idx_sb[:], idx16[:])
            else:
                idx_sb = cpool.tile([P, nt * 2], i32)
                nc.sync.dma_start(idx_sb[:], idx[:])
            z_sb = cpool.tile([P, nt * BL], f32)

            # DVE-side join: absorb the const-load DMA waits once, so no
            # compute instruction ever carries >1 sync wait (HW limit).
            join = cpool.tile([P, 2], f32)
            nc.vector.tensor_copy(join[:, 0:1], v_sb[:, 0:1])
            nc.vector.tensor_copy(join[:, 1:2], bias_sb[:, 0:1])

            # overlapping view of xt: row-pair i = rows (i, i+1) = 2048 elems
            xt_pairs = dataclasses.replace(
                xt[:], ap=[[ROW, NPIX - 1], [1, 2 * ROW]]
            )

            half = 2 * BL * C  # one y-row pair: [x(2), b, c]
            for tg0 in range(0, nt, tg):
                g = gpool.tile([P, tg * 4 * BL * C], bf16, tag="g")
                if gather == "ant":
                    nc.gpsimd.dma_gather(
                        out_ap=g[:].rearrange(
                            "p (s e) -> p s e", s=2 * tg, e=2048
                        ),
                        in_ap=xt_pairs,
                        idxs_ap=idx_sb[:, tg0 * 16 : (tg0 + tg) * 16],
                        num_idxs=tg * 256,
                        num_idxs_reg=tg * 256,
                        elem_size=2048,
                        elem_step=ROW,
                    )
                else:
                    for tt in range(tg):
                        t = tg0 + tt
                        for j in range(2):
                            nc.gpsimd.indirect_dma_start(
                                out=g[:, (2 * tt + j) * half : (2 * tt + j + 1) * half],
                                out_offset=None,
                                in_=xt[:, :],
                                in_offset=bass.IndirectOffsetOnAxis(
                                    ap=idx_sb[:, 2 * t + j : 2 * t + j + 1],
                                    axis=0,
                                ),
                            )
                for tt in range(tg):
                    t = tg0 + tt
                    scr = wpool.tile([P, 2, 2, C], bf16, tag="scr")
                    g_r = g[:].rearrange(
                        "p (tl y x b c) -> p tl y x b c",
                        tl=tg, y=2, x=2, b=BL, c=C,
                    )
                    v_r = v_sb[:, t * 4 * C : (t + 1) * 4 * C].rearrange(
                        "p (y x c) -> p y x c", y=2, x=2, c=C
                    )
                    for bb in range(BL):
                        zcol = z_sb[:, t * BL + bb : t * BL + bb + 1]
                        if compute == "ttr":
                            nc.vector.tensor_tensor_reduce(
                                out=scr[:],
                                in0=g_r[:, tt, :, :, bb, :],
                                in1=v_r,
                                scale=1.0,
                                scalar=0.0,
                                op0=mybir.AluOpType.mult,
                                op1=mybir.AluOpType.add,
                                accum_out=zcol,
                                opt_aps=False,
                            )
                        else:
                            nc.vector.scalar_tensor_tensor(
                                out=scr[:],
                                in0=g_r[:, tt, :, :, bb, :],
                                scalar=1.0,
                                in1=v_r,
                                op0=mybir.AluOpType.mult,
                                op1=mybir.AluOpType.mult,
                                accum_out=zcol,
                            )

            # epilogue: z += bias (broadcast over b)
            ze = cpool.tile([P, nt * BL], f32)
            nc.vector.tensor_tensor(
                out=z_sb[:].rearrange("p (t b) -> p t b", b=BL),
                in0=z_sb[:].rearrange("p (t b) -> p t b", b=BL),
                in1=bias_sb[:].unsqueeze(-1).broadcast_to([P, nt, BL]),
                op=mybir.AluOpType.add,
            )
            # out = exp(min(z,0)) + max(z,0)  == elu(z) + 1
            nc.vector.tensor_scalar_min(ze[:], z_sb[:], 0.0)
            nc.scalar.activation(ze[:], ze[:], mybir.ActivationFunctionType.Exp)
            nc.vector.tensor_scalar_max(z_sb[:], z_sb[:], 0.0)
            nc.vector.tensor_add(z_sb[:], z_sb[:], ze[:])
            nc.sync.dma_start(out[:], z_sb[:])

    # populate .instr bytes for extended-inst InstISA subclasses
    # (dma_gather, tensor_tensor_reduce, load_library); raw Bass skips this
    # Bacc.compile() pass and walrus then fails with "ISA wrong length".
    from concourse.library_overlay import lower_extended_insts
    lower_extended_insts(nc)
    if split_waits:
        _split_multi_waits(nc)
    nc.finalize()
    return nc


def _split_multi_waits(nc):
    """The walrus build in this environment only supports ONE sync-wait slot
    per instruction.  Hoist extra waits onto NoOps inserted just before the
    offending instruction (same engine, so sequencer order enforces them)."""
    import concourse.mybir as mybir
    import bass_rust

    for fn in nc.m.functions:
        for blk in fn.blocks:
            new_insts = []
            for ins in blk.instructions:
                si = getattr(ins, "sync_info", None)
                waits = list(si.on_wait) if si is not None else []
                if len(waits) > 1:
                    for j, w in enumerate(waits[:-1]):
                        nop = mybir.InstNoOp(name=f"{ins.name}-w{j}")
                        nop.engine = ins.engine
                        nop.sync_info = bass_rust.SyncInfo(
                            on_wait=[w], on_update=[]
                        )
                        new_insts.append(nop)
                    ins.sync_info = bass_rust.SyncInfo(
                        on_wait=[waits[-1]], on_update=list(si.on_update)
                    )
                new_insts.append(ins)
            blk.instructions[:] = new_insts


def _host_prep(x, mu, W, b):
    bf16 = ml_dtypes.bfloat16

    # --- per-neuron bilinear indices / weights (shared by all cores) ---
    gx = np.clip(mu[:, 0].astype(np.float64), -1.0, 1.0)
    gy = np.clip(mu[:, 1].astype(np.float64), -1.0, 1.0)
    ix = (gx + 1.0) * (Wd * 0.5) - 0.5
    iy = (gy + 1.0) * (H * 0.5) - 0.5
    x0 = np.floor(ix)
    y0 = np.floor(iy)
    wx1 = (ix - x0).astype(np.float32)
    wy1 = (iy - y0).astype(np.float32)
    wx0 = 1.0 - wx1
    wy0 = 1.0 - wy1
    x0i = np.clip(x0.astype(np.int32), 0, Wd - 2)
    y0i = np.clip(y0.astype(np.int32), 0, H - 2)
    p00 = y0i * Wd + x0i            # row index of (y0, x0); pair covers x0, x0+1
    p01 = p00 + Wd                  # row index of (y1, x0)

    def to_pt(a):  # [N, ...] -> [P, NT, ...] with n = t*128 + p
        return np.ascontiguousarray(
            a.reshape(NT, P, *a.shape[1:]).swapaxes(0, 1)
        )

    idx_np = to_pt(np.stack([p00, p01], axis=-1)).reshape(P, NT * 2)

    # int16 wrapped indices for dma_gather: per gather group of TG tiles,
    # i-order = [t0:p00 x128, t0:p01 x128, t1:p00 x128, ...]; index i lives at
    # [i % 16, i // 16]; replicated across the 8 Q7 core partition groups.
    p00_t = p00.reshape(NT, P)
    p01_t = p01.reshape(NT, P)
    cols = []
    for tg0 in range(0, NT, TG):
        arr = np.concatenate(
            [np.stack([p00_t[t], p01_t[t]]).reshape(-1)
             for t in range(tg0, tg0 + TG)]
        )  # [TG*256] in i-order
        cols.append(arr.reshape(-1, 16).T)  # [16, TG*16]
    idx16_np = np.tile(np.hstack(cols).astype(np.int16), (8, 1))  # [128, NT*16]

    w4_full = np.stack(
        [wx0 * wy0, wx1 * wy0, wx0 * wy1, wx1 * wy1], axis=-1
    ).astype(np.float32)  # [N, 4]
    v_full = (w4_full[:, :, None] * W[:, None, :]).astype(bf16)  # [N, 4, C]
    wv_np = to_pt(v_full).reshape(P, NT * 4 * C)
    biasr_np = to_pt(b.astype(np.float32))  # [P, NT]

    # --- per-core x transpose to pixel-major bf16 ---
    xb = x.astype(bf16).reshape(B, C, NPIX)
    xts = []
    for c in range(NCORES):
        xc = xb[c * BL : (c + 1) * BL]                       # [BL, C, NPIX]
        xt_np = np.ascontiguousarray(xc.transpose(2, 0, 1)).reshape(NPIX, ROW)
        xts.append(xt_np)

    shared = {"wv": wv_np, "idx": idx_np, "idx16": idx16_np, "biasr": biasr_np}
    return [{"xt": xts[c], **shared} for c in range(NCORES)]


def _input_names(nc):
    import concourse.mybir as mybir
    names = set()
    for alloc in nc.m.functions[0].allocations:
        if isinstance(alloc, mybir.MemoryLocationSet) and alloc.kind == "ExternalInput":
            names.add(alloc.memorylocations[0].name)
    return names


def _run(in_maps, trace=False, **kwargs):
    global _PROGRAM
    from concourse import bass_utils

    if _PROGRAM is None:
        _PROGRAM = _build_program()
    want = _input_names(_PROGRAM)
    in_maps = [{k: v for k, v in m.items() if k in want} for m in in_maps]
    rr = bass_utils.run_bass_kernel_spmd(
        _PROGRAM, in_maps, core_ids=list(range(NCORES)), trace=trace, **kwargs
    )
    outs = []
    for c in range(NCORES):
        o = np.asarray(rr.results[c]["out"], dtype=np.float32)  # [P, NT*BL]
        o = o.reshape(P, NT, BL).transpose(2, 1, 0).reshape(BL, N)
        outs.append(o)
    return np.concatenate(outs, axis=0), rr


def kernel(x, mu, W, b):
    in_maps = _host_prep(x, mu, W, b)
    out, _ = _run(in_maps)
    return out
